# revision 60
# baseline (speedup 1.0000x reference)
"""Trainium2 Bass kernel for nn_AttentionBlock (GroupNorm -> QKV -> MHA -> proj -> residual).

Contract: kernel(**inputs) takes the FULL unsharded inputs (numpy), returns the
FULL output. Internally: data-parallel over batch B=8 across 8 NeuronCores, one
batch element per core, single Bass/Tile program run SPMD via
run_bass_kernel_spmd.

v2 design (ACT-exp-bound; PE/DVE/DMA work cut vs the f32 baseline):
  - x streamed in bf16 [128,4,T]; output stored bf16, host converts to f32;
    small f32 consts ride ONE packed DMA so GN starts early
  - GroupNorm rstd via DVE-only bit-trick rsqrt + 2 Newton steps (batched
    across chunks): the ACT engine runs ONLY Exp -> one act-table load
  - qkv/scores in bf16 (scores lhsT=k so S' comes out [s,t]); exp(S'/8) on
    ACT writes fp8e4 es pair-tiles [128,2,T] (adjacent s-chunks)
  - PV runs fp8 DoubleRowSwInterleave (0.5 cyc/col): vt stationary is the
    interleaved 256-column layout [p, pair, head, 128 col-pairs, 2] with
    hosts-side column-reversed wv/bv so psum rows come out in natural order;
    col-pair 63 of ones -> softmax denominator lands in psum row 64 free
    (v carries +bv so PV/den = a + bv and proj needs no bias fixup)
  - den tail per head: copy psum row 64 -> partition 0 (DVE; ACT Copy in the
    endgame when ACT is idle) -> DVE reciprocal_approx -> Pool partition
    broadcast -> DVE normalize mul writing a in fp8 (odd heads write
    partitions 64-127 directly; DVE supports base-shifted outputs)
  - proj: chunks 0-1 as one fp8 SwInterleave matmul per tile (folded early
    with residual x + proj bias into out_acc while pair 3 runs); chunks 2+3
    accumulate in one psum tile at the endgame -> single tail add per tile
  HW notes vs CoreSim: GPSIMD/Pool cannot touch PSUM; plain DoubleRow
  ldweights fails the ISA check (only SwInterleave with a flat 256-wide
  stationary compiles); partition_broadcast reads partition 0 of the TILE,
  not the AP base.
"""

import numpy as np
import ml_dtypes

import concourse.bacc as bacc
import concourse.tile as tile
from concourse import mybir
from concourse.bass_utils import run_bass_kernel_spmd

F32 = mybir.dt.float32
F32R = mybir.dt.float32r
BF16 = mybir.dt.bfloat16
FP8 = mybir.dt.float8e4
I32 = mybir.dt.int32
ALU = mybir.AluOpType
ACTF = mybir.ActivationFunctionType
PM = mybir.MatmulPerfMode

C = 512
T = 1024
NH = 8
CH = 64
GROUPS = 32
EPS = 1e-5
N_CORES = 8
MAGIC = 0x5F3759DF
DEBUG = False


def build_bass():
    nc = bacc.Bacc(
        "TRN2", target_bir_lowering=False, debug=False, enable_asserts=False
    )

    # ---- DRAM I/O (per-core shapes) ----
    d_x = nc.dram_tensor("x", [C, T], BF16, kind="ExternalInput").ap()
    d_wqk = nc.dram_tensor("wqk", [C, 2 * C], BF16, kind="ExternalInput").ap()
    d_wv = nc.dram_tensor("wv", [C, C], BF16, kind="ExternalInput").ap()
    d_wp0 = nc.dram_tensor("wp0", [128, 4, 256], FP8, kind="ExternalInput").ap()
    d_wp1 = nc.dram_tensor("wp1", [128, 2, C], FP8, kind="ExternalInput").ap()
    # packed per-partition f32 smalls: gmat(8) gmt(128) gnw(4) gnb(4) bqk(8) bp(4)
    d_sm = nc.dram_tensor("sm", [128, 156], F32, kind="ExternalInput").ap()
    # vt column-pairs 0..63 prefix (zeros + ones at pair 63), host-supplied
    d_vtz = nc.dram_tensor("vtz", [128, 4, 8, 64, 2], FP8, kind="ExternalInput").ap()
    d_out = nc.dram_tensor("out", [C, T], BF16, kind="ExternalOutput").ap()
    d_dbg = None
    if DEBUG:
        d_dbg = {
            "dbg_gs": nc.dram_tensor("dbg_gs", [8, 4, 2], F32, kind="ExternalOutput").ap(),
            "dbg_y": nc.dram_tensor("dbg_y", [8, 4], F32, kind="ExternalOutput").ap(),
            "dbg_ab": nc.dram_tensor("dbg_ab", [128, 4, 2], F32, kind="ExternalOutput").ap(),
            "dbg_xn": nc.dram_tensor("dbg_xn", [128, 1024], BF16, kind="ExternalOutput").ap(),
            "dbg_qk": nc.dram_tensor("dbg_qk", [128, 1024], BF16, kind="ExternalOutput").ap(),
            "dbg_es": nc.dram_tensor("dbg_es", [128, 2, 1024], mybir.dt.float8e4, kind="ExternalOutput").ap(),
            "dbg_a": nc.dram_tensor("dbg_a", [128, 1024], mybir.dt.float8e4, kind="ExternalOutput").ap(),
        }

    with tile.TileContext(nc) as tc:
        _body(tc, d_x, d_wqk, d_wv, d_wp0, d_wp1, d_sm, d_vtz, d_out, d_dbg)
    nc.compile()
    return nc


def _body(tc, d_x, d_wqk, d_wv, d_wp0, d_wp1, d_sm, d_vtz, d_out, d_dbg=None):
    nc = tc.nc

    with (
        tc.tile_pool(name="persist", bufs=1) as pp,
        tc.tile_pool(name="es", bufs=12) as wk,
        tc.tile_pool(name="tail", bufs=3) as tl,
        tc.tile_pool(name="psum", bufs=3, space="PSUM") as ps,
        tc.tile_pool(name="psacc", bufs=1, space="PSUM") as pa,
    ):
        # ---- persistent SBUF tiles + input DMAs ----
        # one packed DMA for the small f32 tensors (GN consts + biases) so
        # they land ahead of the bulk x/wqk traffic; x and wqk alternate
        # between the two HWDGE rings
        x_sb = pp.tile([128, 4, T], BF16)
        x_r = d_x.rearrange("(j p) t -> p j t", p=128)
        for j in range(4):
            q = nc.sync if j % 2 == 0 else nc.gpsimd
            for sg in range(2):
                q.dma_start(x_sb[:, j, 512 * sg:512 * (sg + 1)],
                            x_r[:, j, 512 * sg:512 * (sg + 1)])
        sm_sb = pp.tile([128, 156], F32)
        nc.scalar.dma_start(sm_sb[:], d_sm[:])
        gmat_sb = sm_sb[:, 0:8]
        gmt_sb = sm_sb[:, 8:136]
        gnw_sb = sm_sb[:, 136:140]
        gnb_sb = sm_sb[:, 140:144]
        bqk_sb = sm_sb[:, 144:152]
        bp_sb = sm_sb[:, 152:156]
        wqk_sb = pp.tile([128, 4, 2 * C], BF16)
        wqk_r = d_wqk.rearrange("(j p) o -> p j o", p=128)
        wv_sb = pp.tile([128, 4, C], BF16)
        wv_r = d_wv.rearrange("(j p) o -> p j o", p=128)
        wp0_sb = pp.tile([128, 4, 256], FP8)
        wp1_sb = pp.tile([128, 2, C], FP8)
        for kc in range(4):
            q = nc.sync if kc % 2 == 0 else nc.gpsimd
            q.dma_start(wqk_sb[:, kc, :], wqk_r[:, kc, :])
        for kc in range(4):
            nc.gpsimd.dma_start(wv_sb[:, kc, :], wv_r[:, kc, :])
        nc.gpsimd.dma_start(wp0_sb[:], d_wp0[:])
        nc.gpsimd.dma_start(wp1_sb[:], d_wp1[:])

        xn_sb = pp.tile([128, 4, T], BF16, tag="xn")
        qk_sb = pp.tile([128, 8, T], BF16)
        # vt interleaved for DoubleRowSwInterleave PV (ldweights requires the
        # full 256-column stationary): [p, pair u, head, 128 column-pairs, i].
        # Column-pair jj maps to psum row 127-jj: jj 64..127 = v channels
        # (host-reversed wv -> natural order), jj 63 = ones (den row 64),
        # jj 0..62 = zeros (psum rows 65..127 unused)
        vt_sb = pp.tile([128, 4, 8, 128, 2], FP8)
        a_sb = pp.tile([128, 4, T], FP8)
        out_acc = pp.tile([128, 4, T], F32)
        out_sb = pp.tile([128, 4, T], BF16)
        out_r = d_out.rearrange("(j p) t -> p j t", p=128)

        # single act-table load (Exp only), pulled off the critical path
        eps_sb = pp.tile([8, 1], F32)
        nc.vector.memset(eps_sb[:], EPS)
        dum_sb = pp.tile([1, 1], F32)
        nc.scalar.activation(dum_sb[:], eps_sb[0:1, 0:1], ACTF.Exp)

        # ================= GroupNorm =================
        stats = pp.tile([128, 4, 2, 6], F32)
        mv = pp.tile([128, 4, 2], F32)
        packed = pp.tile([128, 4, 2], F32)
        msq = pp.tile([128, 4], F32)
        gstats = pp.tile([8, 4, 2], F32)
        msqg = pp.tile([8, 4], F32)
        varg = pp.tile([8, 4], F32)
        bcin = pp.tile([128, 4, 2], F32)
        nc.vector.memset(bcin[:], 0.0)
        iv = pp.tile([8, 4], I32)
        yn = pp.tile([8, 4], F32)
        tn = pp.tile([8, 4], F32)
        A_sb = pp.tile([128, 4], F32)
        t1_sb = pp.tile([128, 4], F32)
        B_sb = pp.tile([128, 4], F32)
        for j in range(4):
            for sg in range(2):
                nc.vector.bn_stats(stats[:, j, sg, :], x_sb[:, j, 512 * sg:512 * (sg + 1)])
            nc.vector.bn_aggr(mv[:, j, :], stats[:, j, :, :])
            nc.vector.tensor_copy(packed[:, j, 0:1], mv[:, j, 0:1])
            nc.vector.tensor_mul(msq[:, j:j + 1], mv[:, j, 0:1], mv[:, j, 0:1])
            nc.vector.tensor_add(packed[:, j, 1:2], msq[:, j:j + 1], mv[:, j, 1:2])
            gp = ps.tile([8, 2], F32, tag="sc", name=f"gn{j}")
            nc.tensor.matmul(gp[:], gmat_sb[:], packed[:, j, :], start=True, stop=True)
            nc.vector.tensor_copy(gstats[:, j, :], gp[:])
        # rstd = rsqrt(var+eps): bit-trick seed + 1 Newton step (error
        # ~2e-3, below the bf16 xn quantization), in two chunk-pair batches
        # so chunks 0-1 unblock while 2-3's stats still stream in
        def rsqrt_batch(j0, j1):
            sl = slice(j0, j1)
            nc.vector.tensor_mul(msqg[:, sl], gstats[:, sl, 0], gstats[:, sl, 0])
            nc.vector.tensor_sub(varg[:, sl], gstats[:, sl, 1], msqg[:, sl])
            nc.vector.tensor_scalar(varg[:, sl], varg[:, sl], EPS, None, op0=ALU.add)
            nc.vector.tensor_scalar(
                iv[:, sl], varg[:, sl].bitcast(I32), 1, None, op0=ALU.arith_shift_right)
            nc.vector.tensor_scalar(iv[:, sl], iv[:, sl], -1, MAGIC, op0=ALU.mult, op1=ALU.add)
            y0 = iv[:, sl].bitcast(F32)
            nc.vector.tensor_mul(tn[:, sl], y0, y0)
            nc.vector.tensor_mul(tn[:, sl], tn[:, sl], varg[:, sl])
            nc.vector.tensor_scalar(tn[:, sl], tn[:, sl], -0.5, 1.5, op0=ALU.mult, op1=ALU.add)
            nc.vector.tensor_mul(yn[:, sl], y0, tn[:, sl])

        def gn_apply(j):
            bb = ps.tile([128, 2], F32, tag="sc", name=f"gb{j}")
            nc.tensor.matmul(bb[:], gmt_sb[:], bcin[:, j, :], start=True, stop=True)
            nc.vector.tensor_mul(A_sb[:, j:j + 1], bb[:, 1:2], gnw_sb[:, j:j + 1])
            # negB = mean*A - gnb; apply = x*A - negB = x*A + B
            nc.vector.scalar_tensor_tensor(
                B_sb[:, j:j + 1], bb[:, 0:1], A_sb[:, j:j + 1],
                gnb_sb[:, j:j + 1], op0=ALU.mult, op1=ALU.subtract)
            nc.vector.tensor_scalar(
                xn_sb[:, j, :], x_sb[:, j, :],
                A_sb[:, j:j + 1], B_sb[:, j:j + 1], op0=ALU.mult, op1=ALU.subtract)

        for j0 in (0, 2):
            rsqrt_batch(j0, j0 + 2)
            sl = slice(j0, j0 + 2)
            nc.vector.tensor_copy(bcin[0:8, sl, 1:2], yn[:, sl])
            nc.vector.tensor_copy(bcin[0:8, sl, 0:1], gstats[:, sl, 0:1])
            gn_apply(j0)
            gn_apply(j0 + 1)

        # ================= QKV / relayout / V =================
        qkq = {}

        def qk_half(jo, th):
            # both t-halves share one [128,1024] psum tile and a single bias
            # op, but are emitted as two fill units so scores interleave
            # between their 4-matmul blocks on the in-order PE
            if th == 0:
                qkq[jo] = ps.tile([128, T], F32, tag="sc", name=f"qk{jo}")
            pq = qkq[jo]
            for kc in range(4):
                nc.tensor.matmul(
                    pq[:, 512 * th:512 * (th + 1)],
                    wqk_sb[:, kc, 128 * jo:128 * (jo + 1)],
                    xn_sb[:, kc, 512 * th:512 * (th + 1)],
                    start=(kc == 0), stop=(kc == 3))
            if th == 1:
                nc.vector.tensor_scalar_add(qk_sb[:, jo, :], pq[:], bqk_sb[:, jo:jo + 1])

        def qk_group(jo):
            qk_half(jo, 0)
            qk_half(jo, 1)

        nc.sync.dma_start(vt_sb[:, :, :, 0:64, :], d_vtz[:])

        def v_group(jt):
            pv_ = ps.tile([128, 512], F32, tag="sc", name=f"v{jt}")
            for kc in range(4):
                nc.tensor.matmul(
                    pv_[:], xn_sb[:, kc, 128 * jt:128 * (jt + 1)],
                    wv_sb[:, kc, :], start=(kc == 0), stop=(kc == 3))
            nc.vector.tensor_copy(
                vt_sb[:, jt // 2, :, 64:128, jt % 2],
                pv_[:].rearrange("p (h c) -> p h c", c=64))

        # ---- proj: fp8 DoubleRow over chunk pairs of a ----
        def proj_u0(jo, th):
            pj = ps.tile([128, 512], F32, tag="sc", name=f"pj0_{jo}_{th}")
            nc.tensor.matmul(
                pj[:], wp0_sb[:, jo, :],
                a_sb[:, 0:2, 512 * th:512 * (th + 1)],
                start=True, stop=True, perf_mode=PM.DoubleRowSwInterleave)
            nc.vector.scalar_tensor_tensor(
                out_acc[:, jo, 512 * th:512 * (th + 1)], pj[:], bp_sb[:, jo:jo + 1],
                x_sb[:, jo, 512 * th:512 * (th + 1)], op0=ALU.add, op1=ALU.add)

        pj23 = {}

        def proj_c23(jo, th, last):
            # chunks 2+3 accumulate in one psum tile; both th halves fold
            # into out_sb with one [128, 1024] add, then one DMA per chunk
            pj = ps.tile([128, 512], F32, tag="sc", name=f"pj3_{jo}_{th}")
            nc.tensor.matmul(
                pj[:], wp1_sb[:, 0, 128 * jo:128 * (jo + 1)],
                a_sb[:, 2, 512 * th:512 * (th + 1)], start=True, stop=False)
            nc.tensor.matmul(
                pj[:], wp1_sb[:, 1, 128 * jo:128 * (jo + 1)],
                a_sb[:, 3, 512 * th:512 * (th + 1)], start=False, stop=True)
            nc.vector.tensor_add(
                out_sb[:, jo, 512 * th:512 * (th + 1)],
                out_acc[:, jo, 512 * th:512 * (th + 1)], pj[:])
            # ACT is idle in the endgame: its HWDGE ring carries half the
            # output stream so the final DMAs don't serialize on one ring
            q_ = nc.sync if jo % 2 == 0 else nc.scalar
            q_.dma_start(out_r[:, jo, 512 * th:512 * (th + 1)],
                         out_sb[:, jo, 512 * th:512 * (th + 1)])

        # ================= attention =================
        def scores(h, sj, sc):
            off, hp = 64 * (h % 2), h // 2
            for th in range(2):
                nc.tensor.matmul(
                    sc[:, 512 * th:512 * (th + 1)],
                    qk_sb[off:off + 64, 4 + hp, 128 * sj:128 * (sj + 1)],
                    qk_sb[off:off + 64, hp, 512 * th:512 * (th + 1)],
                    start=True, stop=True)

        def head_tail(h, pv, nsplit=1):
            hp = h // 2
            den = tl.tile([65, T], F32, tag="den", name=f"dn{h}")
            rden = tl.tile([65, T], F32, tag="rden", name=f"rd{h}")
            bc = tl.tile([64, T], F32, tag="bc", name=f"bc{h}")
            step = T // nsplit
            for s in range(nsplit):
                lo, hi = s * step, (s + 1) * step
                nc.vector.tensor_copy(den[0:1, lo:hi], pv[64:65, lo:hi])
                nc.vector.reciprocal_approx_fast(out=rden[0:1, lo:hi], in_=den[0:1, lo:hi])
                nc.gpsimd.partition_broadcast(bc[:, lo:hi], rden[0:1, lo:hi], channels=64)
                nc.vector.tensor_mul(
                    a_sb[64 * (h % 2):64 * (h % 2) + 64, hp, lo:hi],
                    pv[0:64, lo:hi], bc[:, lo:hi])

        def attn_head(h, fill=None, late=None, pre=None):
            # single-head processing: one PV accumulator live at a time
            # (pa bufs=1 -> 2 banks) buys a third scores slot, so qk/v/proj
            # fill psum holds no longer starve the exp ping-pong. PV runs two
            # chunks behind the exps so the previous head's tail (emitted in
            # `pre`, before this pvt allocation) never stalls it.
            for fn in (pre or []):
                fn()
            pvt = pa.tile([128, T], F32, tag="pv", name=f"pv{h}")
            es = [None] * 4
            fill = list(fill or [])
            late = list(late or [])
            for sj in range(8):
                u, i = sj // 2, sj % 2
                if i == 0:
                    es[u] = wk.tile([128, 2, T], FP8, tag="es", name=f"es{h}_{u}")
                sc = ps.tile([128, T], F32, tag="sc", name=f"sc{h}_{sj}")
                scores(h, sj, sc)
                nc.scalar.activation(es[u][:, i, :], sc[:], ACTF.Exp, scale=0.125)
                if sj >= 4 and i == 0:
                    up = (sj - 4) // 2
                    for th in range(2):
                        nc.tensor.matmul(
                            pvt[:, 512 * th:512 * (th + 1)],
                            vt_sb[:, up, h, :, :].rearrange("p a b -> p (a b)"),
                            es[up][:, :, 512 * th:512 * (th + 1)],
                            start=(up == 0), stop=False,
                            perf_mode=PM.DoubleRowSwInterleave)
                n = -(-len(fill) // (8 - sj)) if fill else 0
                for _ in range(n):
                    fill.pop(0)()
                if sj >= 2 and late:
                    n = -(-len(late) // (8 - sj))
                    for _ in range(n):
                        late.pop(0)()
            for up in (2, 3):
                for th in range(2):
                    nc.tensor.matmul(
                        pvt[:, 512 * th:512 * (th + 1)],
                        vt_sb[:, up, h, :, :].rearrange("p a b -> p (a b)"),
                        es[up][:, :, 512 * th:512 * (th + 1)],
                        start=False, stop=(up == 3),
                        perf_mode=PM.DoubleRowSwInterleave)
            return pvt

        # prelude: head 0/1's q/k chunks
        qk_group(0)
        qk_group(4)

        prev_pvt = None
        for h in range(8):
            fill = []
            late = []
            pre = []
            if h == 0:
                fill += [lambda jt=jt: v_group(jt) for jt in range(8)]
            if h % 2 == 0 and h < 6:
                hp = h // 2
                fill += [lambda jo=jo, th=th: qk_half(jo, th)
                         for jo in (hp + 1, 4 + hp + 1) for th in range(2)]
            if prev_pvt is not None:
                pre += [lambda hh=h - 1, pv=prev_pvt: head_tail(hh, pv, nsplit=1)]
            if h == 5:
                late += [lambda jo=jo, th=th: proj_u0(jo, th)
                         for jo in range(4) for th in range(2)]
            prev_pvt = attn_head(h, fill=fill, late=late, pre=pre)

        # endgame: head 7's tail per t-half (den copies on the idle ACT),
        # chunk-2+3 proj folded per half
        dn7 = tl.tile([65, T], F32, tag="den", name="dn7")
        rd7 = tl.tile([65, T], F32, tag="rden", name="rd7")
        bc7 = tl.tile([64, T], F32, tag="bc", name="bc7e")
        for q in range(2):
            lo, hi = q * 512, (q + 1) * 512
            nc.scalar.activation(dn7[0:1, lo:hi], prev_pvt[64:65, lo:hi], ACTF.Copy)
            nc.vector.reciprocal_approx_fast(out=rd7[0:1, lo:hi], in_=dn7[0:1, lo:hi])
            nc.gpsimd.partition_broadcast(bc7[:, lo:hi], rd7[0:1, lo:hi], channels=64)
            nc.vector.tensor_mul(a_sb[64:128, 3, lo:hi], prev_pvt[0:64, lo:hi],
                                 bc7[:, lo:hi])
            for jo in range(4):
                proj_c23(jo, q, last=(q == 1))


# ----------------------------------------------------------------------------
# Host side
# ----------------------------------------------------------------------------

def make_in_maps(x, norm_w, norm_b, qkv_w, qkv_b, proj_w, proj_b):
    B = x.shape[0]
    bf = ml_dtypes.bfloat16
    f8 = ml_dtypes.float8_e4m3
    xf = np.ascontiguousarray(x.reshape(B, C, T)).astype(bf)
    qkv_w = np.asarray(qkv_w, dtype=np.float32)
    qkv_b = np.asarray(qkv_b, dtype=np.float32)

    q_rows = np.concatenate([np.arange(192 * h, 192 * h + 64) for h in range(NH)])
    k_rows = np.concatenate([np.arange(192 * h + 64, 192 * h + 128) for h in range(NH)])
    v_rows = np.concatenate([np.arange(192 * h + 128, 192 * h + 192) for h in range(NH)])

    wqk = np.empty((C, 2 * C), dtype=np.float32)
    wqk[:, :C] = qkv_w[q_rows].T
    wqk[:, C:] = qkv_w[k_rows].T
    bqk = np.concatenate([qkv_b[q_rows], qkv_b[k_rows]]).astype(np.float32)

    # per-head column-reversed v (cancels the SwInterleave column reversal
    # in the PV stationary, so psum rows come out in natural order)
    wv = np.ascontiguousarray(
        qkv_w[v_rows].T.reshape(C, NH, CH)[:, :, ::-1].reshape(C, C))
    bv = qkv_b[v_rows].astype(np.float32)

    # proj chunks 0-1 in SwInterleave layout:
    #   wp0[p, jo, 2*(127-m)+i] = proj_w[128*jo + m, 128*i + p]
    pw = np.asarray(proj_w, dtype=np.float32)
    wp0 = np.zeros((128, 4, 256), dtype=np.float32)
    for i in range(2):
        for jo in range(4):
            # [m, p] -> [p, m]
            blk = pw[128 * jo:128 * (jo + 1), 128 * i:128 * (i + 1)].T
            wp0[:, jo, (2 * (127 - np.arange(128)) + i)] = blk
    # chunks 2,3 plain: wp1[p, ci, o] = proj_w[o, 256 + 128*ci + p]
    wp1 = np.ascontiguousarray(
        pw.T[256:512].reshape(2, 128, C).transpose(1, 0, 2))
    # v bias folds through the softmax (weights sum to 1) and the projection:
    # out += proj_w @ bv, exactly, so it rides the proj bias instead
    bp = (np.asarray(proj_b, dtype=np.float32) + pw @ bv).astype(np.float32)

    gmat = np.zeros((128, 8), dtype=np.float32)
    for p in range(128):
        gmat[p, p // 16] = 1.0 / 16.0
    gmt = np.zeros((128, 128), dtype=np.float32)
    for p in range(128):
        gmt[p // 16, p] = 1.0

    # packed per-partition f32 smalls: gmat(8) gmt(128) gnw(4) gnb(4) bqk(8) bp(4)
    sm = np.zeros((128, 156), dtype=np.float32)
    sm[:, 0:8] = gmat
    sm[:, 8:136] = gmt
    sm[:, 136:140] = np.asarray(norm_w, dtype=np.float32).reshape(4, 128).T
    sm[:, 140:144] = np.asarray(norm_b, dtype=np.float32).reshape(4, 128).T
    sm[:, 144:152] = bqk.reshape(8, 128).T
    sm[:, 152:156] = bp.reshape(4, 128).T
    vtz = np.zeros((128, 4, 8, 64, 2), dtype=np.float32)
    vtz[:, :, :, 63, :] = 1.0

    shared = dict(
        wqk=wqk.astype(bf), wv=wv.astype(bf), wp0=wp0.astype(f8),
        wp1=wp1.astype(f8), sm=sm, vtz=vtz.astype(f8),
    )
    return [dict(shared, x=xf[b]) for b in range(B)]


_NC_CACHE = []


def _get_nc():
    if not _NC_CACHE:
        _NC_CACHE.append(build_bass())
    return _NC_CACHE[0]


def kernel(x, norm_w, norm_b, qkv_w, qkv_b, proj_w, proj_b):
    x = np.asarray(x)
    B, _, H, W = x.shape
    in_maps = make_in_maps(x, norm_w, norm_b, qkv_w, qkv_b, proj_w, proj_b)
    nc = _get_nc()
    res = run_bass_kernel_spmd(nc, in_maps, core_ids=list(range(N_CORES)))
    out = np.stack([
        np.asarray(res.results[b]["out"]).astype(np.float32).reshape(C, H, W)
        for b in range(B)
    ])
    return out
